# revision 1
# baseline (speedup 1.0000x reference)
"""ComplexAttentionV3 Trainium2 kernel (v2).

Sharding: 8 cores = data-parallel over batch (2) x tensor-parallel over
heads (16 -> 4 per core). Each core computes q/k/v for its 4 heads
(column-sharded projections), local attention, and a row-sharded
o-projection producing a partial [T, D] output; the host sums the 4
partials per batch.

v2 notes vs v1: input DMAs split across both HWDGE queues (SP + ACT) and
ordered so the PE can start within ~1 MB of traffic; q/k projections run
before v (they consume x incrementally); attention uses 2-bank-wide PSUM
tiles so exp/reciprocal/normalize run half as many, twice as large ops;
v-projection streams a packed [wvr | wvi] rhs (halves its matmul count);
softmax reciprocal uses the fast custom-DVE approximation straight out
of PSUM; PSUM pools are per-phase and released between phases.
"""

import numpy as np
import ml_dtypes

import concourse.bacc as bacc
import concourse.tile as tile
from concourse import mybir
from concourse.bass import ts
from concourse.bass_utils import run_bass_kernel_spmd

B, T, D, H = 2, 2048, 1024, 16
HD = 64
NCORE = 8
TP = 4               # head-parallel degree (per batch)
HC = H // TP         # heads per core = 4
C = HC * HD          # local channels = 256
DC = D // 128        # contraction chunks = 8
TQ = T // 128        # 128-row t-chunks = 16
T5 = T // 512        # 512-col t-chunks = 4
TW = T // 1024       # 1024-col t-chunks = 2

F32 = mybir.dt.float32
BF16 = mybir.dt.bfloat16
EXP = mybir.ActivationFunctionType.Exp

LAST_RESULTS = None
_COMPILED = None


def _build():
    nc = bacc.Bacc("TRN2", target_bir_lowering=False, debug=False,
                   num_devices=NCORE)

    def din(name, shape, dt=BF16):
        return nc.dram_tensor(name, shape, dt, kind="ExternalInput").ap()

    xr_d = din("xrT", [128, DC, T])
    xi_d = din("xiT", [128, DC, T])
    wq = {k: din(f"wq_{k}", [128, DC, C]) for k in ("r", "i", "n")}
    wk = {k: din(f"wk_{k}", [128, DC, C]) for k in ("r", "i", "n")}
    wv = {k: din(f"wv_{k}", [128, DC, 2 * C]) for k in ("a", "b")}
    ow = {k: din(f"ow_{k}", [128, 2, D]) for k in ("r", "i", "n")}
    cos_d = din("cos2", [128, T], F32)
    sin_d = din("sin2", [128, T], F32)
    outr_d = nc.dram_tensor("out_r", [T, D], F32, kind="ExternalOutput").ap()
    outi_d = nc.dram_tensor("out_i", [T, D], F32, kind="ExternalOutput").ap()

    with tile.TileContext(nc) as tc:
        with tc.tile_pool(name="persist", bufs=1) as persist:
            qkcat = persist.tile([128, 2 * HC, T], BF16, name="qkcat")
            vcat = persist.tile([128, TQ, HC, 128], BF16, name="vcat")
            urt = persist.tile([128, 2, T], BF16, name="urt")
            uit = persist.tile([128, 2, T], BF16, name="uit")
            ones = persist.tile([128, 1], BF16, name="ones")
            nc.vector.memset(ones[:], 1.0)

            # ---------------- projection phase ----------------
            with tc.tile_pool(name="xw", bufs=1) as xw, \
                 tc.tile_pool(name="rt", bufs=1) as rt, \
                 tc.tile_pool(name="pp", bufs=2, space="PSUM") as pp:
                # weights + rope tables ride the ACT HWDGE queue, x rides
                # SP: the first q matmul needs only wq + xr[dc0].
                wqs = {k: xw.tile([128, DC, C], BF16, name=f"wq{k}")
                       for k in ("r", "i", "n")}
                wks = {k: xw.tile([128, DC, C], BF16, name=f"wk{k}")
                       for k in ("r", "i", "n")}
                wvs = {k: xw.tile([128, DC, 2 * C], BF16, name=f"wv{k}")
                       for k in ("a", "b")}
                cos = xw.tile([128, T], F32, name="cos")
                sin = xw.tile([128, T], F32, name="sin")
                for k in ("r", "i", "n"):
                    nc.scalar.dma_start(wqs[k][:], wq[k][:])
                    nc.scalar.dma_start(wks[k][:], wk[k][:])
                for k in ("a", "b"):
                    nc.scalar.dma_start(wvs[k][:], wv[k][:])
                nc.scalar.dma_start(cos[:], cos_d[:])
                nc.scalar.dma_start(sin[:], sin_d[:])
                xr = xw.tile([128, DC, T], BF16, name="xr")
                xi = xw.tile([128, DC, T], BF16, name="xi")
                for dc in range(DC):
                    nc.sync.dma_start(xr[:, dc, :], xr_d[:, dc, :])
                    nc.sync.dma_start(xi[:, dc, :], xi_d[:, dc, :])

                # q/k projections (transposed [c, t]) + RoPE into qkcat
                for wsrc, hbase in ((wqs, 0), (wks, HC)):
                    for cc in range(2):
                        h0, h1 = hbase + 2 * cc, hbase + 2 * cc + 1
                        for tw in range(TW):
                            pqr = pp.tile([128, 1024], F32, name="ppa")
                            pqi = pp.tile([128, 1024], F32, name="ppb")
                            for half in range(2):
                                hsl = ts(2 * tw + half, 512)
                                psl = ts(half, 512)
                                for dc in range(DC):
                                    nc.tensor.matmul(
                                        pqr[:, psl],
                                        lhsT=wsrc["r"][:, dc, ts(cc, 128)],
                                        rhs=xr[:, dc, hsl],
                                        start=(dc == 0), stop=False)
                                for dc in range(DC):
                                    nc.tensor.matmul(
                                        pqr[:, psl],
                                        lhsT=wsrc["n"][:, dc, ts(cc, 128)],
                                        rhs=xi[:, dc, hsl],
                                        start=False, stop=(dc == DC - 1))
                                for dc in range(DC):
                                    nc.tensor.matmul(
                                        pqi[:, psl],
                                        lhsT=wsrc["i"][:, dc, ts(cc, 128)],
                                        rhs=xr[:, dc, hsl],
                                        start=(dc == 0), stop=False)
                                for dc in range(DC):
                                    nc.tensor.matmul(
                                        pqi[:, psl],
                                        lhsT=wsrc["r"][:, dc, ts(cc, 128)],
                                        rhs=xi[:, dc, hsl],
                                        start=False, stop=(dc == DC - 1))
                            tsl = ts(tw, 1024)
                            t1 = rt.tile([128, 1024], F32, name="t1")
                            t2 = rt.tile([128, 1024], F32, name="t2")
                            t3 = rt.tile([128, 1024], F32, name="t3")
                            t4 = rt.tile([128, 1024], F32, name="t4")
                            nc.vector.tensor_mul(t1[:], pqr[:], cos[:, tsl])
                            nc.vector.tensor_mul(t2[:], pqi[:], sin[:, tsl])
                            nc.vector.tensor_mul(t3[:], pqr[:], sin[:, tsl])
                            nc.vector.tensor_mul(t4[:], pqi[:], cos[:, tsl])
                            nc.vector.tensor_sub(qkcat[0:64, h0, tsl],
                                                 t1[0:64, :], t2[0:64, :])
                            nc.vector.tensor_sub(qkcat[0:64, h1, tsl],
                                                 t1[64:128, :], t2[64:128, :])
                            nc.vector.tensor_add(qkcat[64:128, h0, tsl],
                                                 t3[0:64, :], t4[0:64, :])
                            nc.vector.tensor_add(qkcat[64:128, h1, tsl],
                                                 t3[64:128, :], t4[64:128, :])

                # v projection: natural [t, c], rhs packed [wvr | wvi]
                for tq in range(TQ):
                    pv = pp.tile([128, 1024], F32, name="ppa")
                    pvs = pv[:, 0:512]
                    for dc in range(DC):
                        nc.tensor.matmul(pvs, lhsT=xr[:, dc, ts(tq, 128)],
                                         rhs=wvs["a"][:, dc, :],
                                         start=(dc == 0), stop=False)
                    for dc in range(DC):
                        nc.tensor.matmul(pvs, lhsT=xi[:, dc, ts(tq, 128)],
                                         rhs=wvs["b"][:, dc, :],
                                         start=False, stop=(dc == DC - 1))
                    nc.scalar.copy(
                        vcat[:, tq, :, 0:64],
                        pv[:, 0:C].rearrange("p (h d) -> p h d", h=HC))
                    nc.scalar.copy(
                        vcat[:, tq, :, 64:128],
                        pv[:, C:2 * C].rearrange("p (h d) -> p h d", h=HC))

            # ---------------- attention phase ----------------
            with tc.tile_pool(name="att", bufs=4) as att, \
                 tc.tile_pool(name="attsm", bufs=2) as attsm, \
                 tc.tile_pool(name="sp", bufs=2, space="PSUM") as sp, \
                 tc.tile_pool(name="avp", bufs=1, space="PSUM") as avp, \
                 tc.tile_pool(name="dp", bufs=1, space="PSUM") as dp:
                for h in range(HC):
                    ucc, up0 = h // 2, (h % 2) * 64
                    for iw in range(TW):
                        isl = ts(iw, 1024)
                        av = avp.tile([128, 1024], F32, name="av")
                        dn = dp.tile([1, 1024], F32, name="dn")
                        for jc in range(TQ):
                            s = sp.tile([128, 1024], F32, name="s")
                            for half in range(2):
                                psl = ts(half, 512)
                                nc.tensor.matmul(
                                    s[:, psl],
                                    lhsT=qkcat[:, HC + h, ts(jc, 128)],
                                    rhs=qkcat[:, h, ts(2 * iw + half, 512)],
                                    start=True, stop=True)
                            es = att.tile([128, 1024], BF16, name="es")
                            nc.scalar.activation(es[:], s[:], EXP, scale=0.125)
                            for half in range(2):
                                psl = ts(half, 512)
                                nc.tensor.matmul(av[:, psl],
                                                 lhsT=vcat[:, jc, h, :],
                                                 rhs=es[:, psl],
                                                 start=(jc == 0),
                                                 stop=(jc == TQ - 1))
                                nc.tensor.matmul(dn[:, psl], lhsT=ones[:],
                                                 rhs=es[:, psl],
                                                 start=(jc == 0),
                                                 stop=(jc == TQ - 1))
                        rec = attsm.tile([1, 1024], F32, name="rec")
                        nc.vector.reciprocal_approx_fast(rec[:], dn[:])
                        bc = attsm.tile([128, 1024], F32, name="bc")
                        nc.gpsimd.partition_broadcast(bc[:], rec[:])
                        nc.vector.tensor_mul(urt[up0:up0 + 64, ucc, isl],
                                             av[0:64, :], bc[0:64, :])
                        nc.vector.tensor_mul(uit[up0:up0 + 64, ucc, isl],
                                             av[64:128, :], bc[64:128, :])

            # ---------------- output projection ----------------
            with tc.tile_pool(name="ox", bufs=1) as ox, \
                 tc.tile_pool(name="ost", bufs=3) as ost, \
                 tc.tile_pool(name="op", bufs=2, space="PSUM") as op:
                ows = {k: ox.tile([128, 2, D], BF16, name=f"ow{k}")
                       for k in ("r", "i", "n")}
                for k in ("r", "i", "n"):
                    nc.scalar.dma_start(ows[k][:], ow[k][:])
                for tq in range(TQ):
                    tslq = ts(tq, 128)
                    por = op.tile([128, 1024], F32, name="opa")
                    poi = op.tile([128, 1024], F32, name="opb")
                    for oc in range(2):
                        osl = ts(oc, 512)
                        nc.tensor.matmul(por[:, osl], lhsT=urt[:, 0, tslq],
                                         rhs=ows["r"][:, 0, osl],
                                         start=True, stop=False)
                        nc.tensor.matmul(por[:, osl], lhsT=urt[:, 1, tslq],
                                         rhs=ows["r"][:, 1, osl],
                                         start=False, stop=False)
                        nc.tensor.matmul(por[:, osl], lhsT=uit[:, 0, tslq],
                                         rhs=ows["n"][:, 0, osl],
                                         start=False, stop=False)
                        nc.tensor.matmul(por[:, osl], lhsT=uit[:, 1, tslq],
                                         rhs=ows["n"][:, 1, osl],
                                         start=False, stop=True)
                        nc.tensor.matmul(poi[:, osl], lhsT=urt[:, 0, tslq],
                                         rhs=ows["i"][:, 0, osl],
                                         start=True, stop=False)
                        nc.tensor.matmul(poi[:, osl], lhsT=urt[:, 1, tslq],
                                         rhs=ows["i"][:, 1, osl],
                                         start=False, stop=False)
                        nc.tensor.matmul(poi[:, osl], lhsT=uit[:, 0, tslq],
                                         rhs=ows["r"][:, 0, osl],
                                         start=False, stop=False)
                        nc.tensor.matmul(poi[:, osl], lhsT=uit[:, 1, tslq],
                                         rhs=ows["r"][:, 1, osl],
                                         start=False, stop=True)
                    st = ost.tile([128, 1024], F32, name="st")
                    nc.scalar.copy(st[:], por[:])
                    nc.sync.dma_start(outr_d[tslq, :], st[:])
                    sti = ost.tile([128, 1024], F32, name="sti")
                    nc.scalar.copy(sti[:], poi[:])
                    nc.sync.dma_start(outi_d[tslq, :], sti[:])

    nc.compile()
    return nc


def _to_bf16_kxm(arr, parts=128):
    """[K, M] fp32 -> [128, K//128, M] bf16 with K split as (chunk, part)."""
    k, m = arr.shape
    out = arr.reshape(k // parts, parts, m).transpose(1, 0, 2)
    return np.ascontiguousarray(out.astype(ml_dtypes.bfloat16))


def _rope_tables():
    inv_freq = 1.0 / (10000.0 ** (np.arange(0, HD, 2, dtype=np.float64) / HD))
    invf64 = np.concatenate([inv_freq, inv_freq])          # [64]
    ang = invf64[:, None] * np.arange(T, dtype=np.float64)[None, :]  # [64, T]
    cos2 = np.tile(np.cos(ang), (2, 1)).astype(np.float32)
    sin2 = np.tile(np.sin(ang), (2, 1)).astype(np.float32)
    return np.ascontiguousarray(cos2), np.ascontiguousarray(sin2)


def kernel(x_real, x_imag, q_wr, q_wi, k_wr, k_wi, v_wr, v_wi, o_wr, o_wi):
    global _COMPILED, LAST_RESULTS
    if _COMPILED is None:
        _COMPILED = _build()
    nc = _COMPILED

    cos2, sin2 = _rope_tables()
    xt = {}
    for b in range(B):
        xt[("r", b)] = _to_bf16_kxm(np.asarray(x_real[b]).T.astype(np.float32))
        xt[("i", b)] = _to_bf16_kxm(np.asarray(x_imag[b]).T.astype(np.float32))

    in_maps = []
    for core in range(NCORE):
        b, g = core // TP, core % TP
        cols = slice(g * C, (g + 1) * C)
        m = {"xrT": xt[("r", b)], "xiT": xt[("i", b)],
             "cos2": cos2, "sin2": sin2}
        for nm, wr_, wi_ in (("wq", q_wr, q_wi), ("wk", k_wr, k_wi)):
            m[f"{nm}_r"] = _to_bf16_kxm(np.asarray(wr_[:, cols]))
            m[f"{nm}_i"] = _to_bf16_kxm(np.asarray(wi_[:, cols]))
            m[f"{nm}_n"] = _to_bf16_kxm(-np.asarray(wi_[:, cols]))
        vr_, vi_ = np.asarray(v_wr[:, cols]), np.asarray(v_wi[:, cols])
        m["wv_a"] = _to_bf16_kxm(np.concatenate([vr_, vi_], axis=1))
        m["wv_b"] = _to_bf16_kxm(np.concatenate([-vi_, vr_], axis=1))
        m["ow_r"] = _to_bf16_kxm(np.asarray(o_wr[cols, :]))
        m["ow_i"] = _to_bf16_kxm(np.asarray(o_wi[cols, :]))
        m["ow_n"] = _to_bf16_kxm(-np.asarray(o_wi[cols, :]))
        in_maps.append(m)

    res = run_bass_kernel_spmd(nc, in_maps, core_ids=list(range(NCORE)))
    LAST_RESULTS = res

    final_r = np.zeros((B, T, D), np.float32)
    final_i = np.zeros((B, T, D), np.float32)
    for core in range(NCORE):
        b = core // TP
        final_r[b] += res.results[core]["out_r"]
        final_i[b] += res.results[core]["out_i"]
    return final_r, final_i



# revision 9
# speedup vs baseline: 1.2113x; 1.2113x over previous
"""ComplexAttentionV3 Trainium2 kernel (v3).

Sharding: 8 cores = data-parallel over batch (2) x tensor-parallel over
heads (16 -> 4 per core). Each core computes q/k/v for its 4 heads
(column-sharded projections), local attention, and a row-sharded
o-projection producing a partial [T, D] output; the host sums the 4
partials per batch.

v3 notes vs v2 (559us baseline):
- attention software-pipelined: scores for pair p+1 are emitted before
  the AV matmuls of pair p, so the PE never idles waiting for the ACT
  exp (the 822ns/2jc stall + the resulting p-state down-clock were the
  dominant cost in v2's 272us attention phase);
- softmax denominator fused into the AV matmul as a 65th lhsT column of
  ones (v_real | 1), removing the separate dn matmuls' PSUM pool and
  freeing banks for double-buffered accumulators (av pools bufs=2), so
  the per-(h,iw) normalize chain (DVE recip + gpsimd broadcast + muls)
  runs off the PE critical path;
- attention works on 512-query windows: scores pair tile [128,1024]
  (2 banks, bufs=2) + avr [65,512] + avi [64,512] (1 bank, bufs=2 each)
  = 8 PSUM banks exactly;
- o-proj weights DMA'd at attention start (v2 left them to the o-proj
  phase and stalled 10us); outputs alternate between the sync and
  scalar HWDGE queues (v2 pushed all 16MB through one queue);
- x DMA'd in 512-column slabs interleaved r/i to match first-use order.
"""

import numpy as np
import ml_dtypes

import concourse.bacc as bacc
import concourse.tile as tile
from concourse import mybir
from concourse.bass import ts
from concourse.bass_utils import run_bass_kernel_spmd

B, T, D, H = 2, 2048, 1024, 16
HD = 64
NCORE = 8
TP = 4               # head-parallel degree (per batch)
HC = H // TP         # heads per core = 4
C = HC * HD          # local channels = 256
DC = D // 128        # contraction chunks = 8
TQ = T // 128        # 128-row t-chunks = 16
T5 = T // 512        # 512-col t-chunks = 4
TW = T // 1024       # 1024-col t-chunks = 2
NP = TQ // 2         # key-chunk pairs = 8

F32 = mybir.dt.float32
BF16 = mybir.dt.bfloat16
EXP = mybir.ActivationFunctionType.Exp

LAST_RESULTS = None
_COMPILED = None


def _build():
    nc = bacc.Bacc("TRN2", target_bir_lowering=False, debug=False,
                   num_devices=NCORE)

    def din(name, shape, dt=BF16):
        return nc.dram_tensor(name, shape, dt, kind="ExternalInput").ap()

    xr_d = din("xrT", [128, DC, T])
    xi_d = din("xiT", [128, DC, T])
    wq = {k: din(f"wq_{k}", [128, DC, C]) for k in ("r", "i", "n")}
    wk = {k: din(f"wk_{k}", [128, DC, C]) for k in ("r", "i", "n")}
    wv = {k: din(f"wv_{k}", [128, DC, 2 * C]) for k in ("a", "b")}
    ow = {k: din(f"ow_{k}", [128, 2, D]) for k in ("r", "i", "n")}
    cos_d = din("cos2", [128, T], F32)
    sin_d = din("sin2", [128, T], F32)
    outr_d = nc.dram_tensor("out_r", [T, D], F32, kind="ExternalOutput").ap()
    outi_d = nc.dram_tensor("out_i", [T, D], F32, kind="ExternalOutput").ap()

    with tile.TileContext(nc) as tc:
        with tc.tile_pool(name="persist", bufs=1) as persist:
            qkcat = persist.tile([128, 2 * HC, T], BF16, name="qkcat")
            # v_real packed with a ones column per (key-chunk, head) for the
            # fused softmax-denominator row; v_imag separate. Flat index is
            # tq * HC + h.
            vcr = persist.tile([128, TQ * HC, 65], BF16, name="vcr")
            vci = persist.tile([128, TQ * HC, 64], BF16, name="vci")
            urt = persist.tile([128, 2, T], BF16, name="urt")
            uit = persist.tile([128, 2, T], BF16, name="uit")
            nc.vector.memset(vcr[:, :, 64:65], 1.0)

            # ---------------- projection phase ----------------
            with tc.tile_pool(name="xw", bufs=1) as xw, \
                 tc.tile_pool(name="rt", bufs=1) as rt, \
                 tc.tile_pool(name="pp", bufs=2, space="PSUM") as pp:
                # scalar HWDGE queue: weights + rope tables in first-use
                # order; sync HWDGE queue: x in 512-col slabs, r/i
                # interleaved (q-proj consumes slab s of xr then xi).
                wqs = {k: xw.tile([128, DC, C], BF16, name=f"wq{k}")
                       for k in ("r", "i", "n")}
                wks = {k: xw.tile([128, DC, C], BF16, name=f"wk{k}")
                       for k in ("r", "i", "n")}
                wvs = {k: xw.tile([128, DC, 2 * C], BF16, name=f"wv{k}")
                       for k in ("a", "b")}
                cos = xw.tile([128, T], F32, name="cos")
                sin = xw.tile([128, T], F32, name="sin")
                for k in ("r", "i", "n"):
                    nc.scalar.dma_start(wqs[k][:], wq[k][:])
                nc.scalar.dma_start(cos[:], cos_d[:])
                nc.scalar.dma_start(sin[:], sin_d[:])
                for k in ("r", "i", "n"):
                    nc.scalar.dma_start(wks[k][:], wk[k][:])
                for k in ("a", "b"):
                    nc.scalar.dma_start(wvs[k][:], wv[k][:])
                xr = xw.tile([128, DC, T], BF16, name="xr")
                xi = xw.tile([128, DC, T], BF16, name="xi")
                for sl in range(T5):
                    ssl = ts(sl, 512)
                    nc.sync.dma_start(xr[:, :, ssl], xr_d[:, :, ssl])
                    nc.sync.dma_start(xi[:, :, ssl], xi_d[:, :, ssl])

                # q/k projections (transposed [c, t]) + RoPE into qkcat
                for wsrc, hbase in ((wqs, 0), (wks, HC)):
                    for cc in range(2):
                        h0, h1 = hbase + 2 * cc, hbase + 2 * cc + 1
                        for tw in range(TW):
                            pqr = pp.tile([128, 1024], F32, name="ppa")
                            pqi = pp.tile([128, 1024], F32, name="ppb")
                            for half in range(2):
                                hsl = ts(2 * tw + half, 512)
                                psl = ts(half, 512)
                                for dc in range(DC):
                                    nc.tensor.matmul(
                                        pqr[:, psl],
                                        lhsT=wsrc["r"][:, dc, ts(cc, 128)],
                                        rhs=xr[:, dc, hsl],
                                        start=(dc == 0), stop=False)
                                for dc in range(DC):
                                    nc.tensor.matmul(
                                        pqr[:, psl],
                                        lhsT=wsrc["n"][:, dc, ts(cc, 128)],
                                        rhs=xi[:, dc, hsl],
                                        start=False, stop=(dc == DC - 1))
                                for dc in range(DC):
                                    nc.tensor.matmul(
                                        pqi[:, psl],
                                        lhsT=wsrc["i"][:, dc, ts(cc, 128)],
                                        rhs=xr[:, dc, hsl],
                                        start=(dc == 0), stop=False)
                                for dc in range(DC):
                                    nc.tensor.matmul(
                                        pqi[:, psl],
                                        lhsT=wsrc["r"][:, dc, ts(cc, 128)],
                                        rhs=xi[:, dc, hsl],
                                        start=False, stop=(dc == DC - 1))
                            tsl = ts(tw, 1024)
                            t1 = rt.tile([128, 1024], F32, name="t1")
                            t2 = rt.tile([128, 1024], F32, name="t2")
                            t3 = rt.tile([128, 1024], F32, name="t3")
                            t4 = rt.tile([128, 1024], F32, name="t4")
                            nc.vector.tensor_mul(t1[:], pqr[:], cos[:, tsl])
                            nc.vector.tensor_mul(t2[:], pqi[:], sin[:, tsl])
                            nc.vector.tensor_mul(t3[:], pqr[:], sin[:, tsl])
                            nc.vector.tensor_mul(t4[:], pqi[:], cos[:, tsl])
                            nc.vector.tensor_sub(qkcat[0:64, h0, tsl],
                                                 t1[0:64, :], t2[0:64, :])
                            nc.vector.tensor_sub(qkcat[0:64, h1, tsl],
                                                 t1[64:128, :], t2[64:128, :])
                            nc.vector.tensor_add(qkcat[64:128, h0, tsl],
                                                 t3[0:64, :], t4[0:64, :])
                            nc.vector.tensor_add(qkcat[64:128, h1, tsl],
                                                 t3[64:128, :], t4[64:128, :])

                # v projection: natural [t, c], rhs packed [wvr | wvi]
                for tq in range(TQ):
                    pv = pp.tile([128, 1024], F32, name="ppa")
                    pvs = pv[:, 0:512]
                    for dc in range(DC):
                        nc.tensor.matmul(pvs, lhsT=xr[:, dc, ts(tq, 128)],
                                         rhs=wvs["a"][:, dc, :],
                                         start=(dc == 0), stop=False)
                    for dc in range(DC):
                        nc.tensor.matmul(pvs, lhsT=xi[:, dc, ts(tq, 128)],
                                         rhs=wvs["b"][:, dc, :],
                                         start=False, stop=(dc == DC - 1))
                    nc.scalar.copy(
                        vcr[:, tq * HC:(tq + 1) * HC, 0:64],
                        pv[:, 0:C].rearrange("p (h d) -> p h d", h=HC))
                    nc.scalar.copy(
                        vci[:, tq * HC:(tq + 1) * HC, :],
                        pv[:, C:2 * C].rearrange("p (h d) -> p h d", h=HC))

            # ---------------- attention phase ----------------
            # per (head, 512-query window): 8 key-chunk pairs; scores for
            # pair p+1 are emitted before the AV matmuls of pair p so the
            # exp latency is hidden behind ~1.3us of PE work.
            with tc.tile_pool(name="ox", bufs=1) as ox:
                # prefetch o-proj weights now: the scalar queue is idle and
                # SBUF has room once the projection pools wind down.
                ows = {k: ox.tile([128, 2, D], BF16, name=f"ow{k}")
                       for k in ("r", "i", "n")}
                for k in ("r", "i", "n"):
                    nc.scalar.dma_start(ows[k][:], ow[k][:])

                with tc.tile_pool(name="att", bufs=3) as att, \
                     tc.tile_pool(name="attsm", bufs=2) as attsm, \
                     tc.tile_pool(name="sp", bufs=2, space="PSUM") as sp, \
                     tc.tile_pool(name="avr", bufs=2, space="PSUM") as avrp, \
                     tc.tile_pool(name="avi", bufs=2, space="PSUM") as avip:
                    for h in range(HC):
                        ucc, up0 = h // 2, (h % 2) * 64
                        for iw in range(T5):
                            isl = ts(iw, 512)
                            qsl = qkcat[:, h, isl]
                            avr = avrp.tile([65, 512], F32, name="avr")
                            avi = avip.tile([64, 512], F32, name="avi")

                            def scores(p, etiles, h=h, qsl=qsl):
                                s = sp.tile([128, 1024], F32, name="s")
                                for j in range(2):
                                    nc.tensor.matmul(
                                        s[:, ts(j, 512)],
                                        lhsT=qkcat[:, HC + h,
                                                   ts(2 * p + j, 128)],
                                        rhs=qsl, start=True, stop=True)
                                es = att.tile([128, 1024], BF16, name="es")
                                nc.scalar.activation(es[:], s[:], EXP,
                                                     scale=0.125)
                                etiles[p] = es

                            def accum(p, etiles, h=h, avr=avr, avi=avi):
                                es = etiles.pop(p)
                                for j in range(2):
                                    nc.tensor.matmul(
                                        avr[:],
                                        lhsT=vcr[:, (2 * p + j) * HC + h, :],
                                        rhs=es[:, ts(j, 512)],
                                        start=(p == 0 and j == 0),
                                        stop=(p == NP - 1 and j == 1))
                                for j in range(2):
                                    nc.tensor.matmul(
                                        avi[:],
                                        lhsT=vci[:, (2 * p + j) * HC + h, :],
                                        rhs=es[:, ts(j, 512)],
                                        start=(p == 0 and j == 0),
                                        stop=(p == NP - 1 and j == 1))

                            etiles = {}
                            scores(0, etiles)
                            for p in range(NP):
                                if p + 1 < NP:
                                    scores(p + 1, etiles)
                                accum(p, etiles)

                            dnr = attsm.tile([1, 512], F32, name="dnr")
                            nc.scalar.copy(dnr[:], avr[64:65, :])
                            rec = attsm.tile([1, 512], F32, name="rec")
                            nc.vector.reciprocal_approx_fast(rec[:], dnr[:])
                            bc = attsm.tile([128, 512], F32, name="bc")
                            nc.gpsimd.partition_broadcast(bc[:], rec[:])
                            nc.vector.tensor_mul(urt[up0:up0 + 64, ucc, isl],
                                                 avr[0:64, :], bc[0:64, :])
                            nc.vector.tensor_mul(uit[up0:up0 + 64, ucc, isl],
                                                 avi[0:64, :], bc[64:128, :])

                # ---------------- output projection ----------------
                with tc.tile_pool(name="ost", bufs=3) as ost, \
                     tc.tile_pool(name="op", bufs=2, space="PSUM") as op:
                    for tq in range(TQ):
                        tslq = ts(tq, 128)
                        por = op.tile([128, 1024], F32, name="opa")
                        poi = op.tile([128, 1024], F32, name="opb")
                        for oc in range(2):
                            osl = ts(oc, 512)
                            nc.tensor.matmul(por[:, osl], lhsT=urt[:, 0, tslq],
                                             rhs=ows["r"][:, 0, osl],
                                             start=True, stop=False)
                            nc.tensor.matmul(por[:, osl], lhsT=urt[:, 1, tslq],
                                             rhs=ows["r"][:, 1, osl],
                                             start=False, stop=False)
                            nc.tensor.matmul(por[:, osl], lhsT=uit[:, 0, tslq],
                                             rhs=ows["n"][:, 0, osl],
                                             start=False, stop=False)
                            nc.tensor.matmul(por[:, osl], lhsT=uit[:, 1, tslq],
                                             rhs=ows["n"][:, 1, osl],
                                             start=False, stop=True)
                            nc.tensor.matmul(poi[:, osl], lhsT=urt[:, 0, tslq],
                                             rhs=ows["i"][:, 0, osl],
                                             start=True, stop=False)
                            nc.tensor.matmul(poi[:, osl], lhsT=urt[:, 1, tslq],
                                             rhs=ows["i"][:, 1, osl],
                                             start=False, stop=False)
                            nc.tensor.matmul(poi[:, osl], lhsT=uit[:, 0, tslq],
                                             rhs=ows["r"][:, 0, osl],
                                             start=False, stop=False)
                            nc.tensor.matmul(poi[:, osl], lhsT=uit[:, 1, tslq],
                                             rhs=ows["r"][:, 1, osl],
                                             start=False, stop=True)
                        st = ost.tile([128, 1024], F32, name="st")
                        nc.scalar.copy(st[:], por[:])
                        nc.sync.dma_start(outr_d[tslq, :], st[:])
                        sti = ost.tile([128, 1024], F32, name="sti")
                        nc.vector.tensor_copy(sti[:], poi[:])
                        nc.scalar.dma_start(outi_d[tslq, :], sti[:])

    nc.compile()
    return nc


def _to_bf16_kxm(arr, parts=128):
    """[K, M] fp32 -> [128, K//128, M] bf16 with K split as (chunk, part)."""
    k, m = arr.shape
    out = arr.reshape(k // parts, parts, m).transpose(1, 0, 2)
    return np.ascontiguousarray(out.astype(ml_dtypes.bfloat16))


def _rope_tables():
    inv_freq = 1.0 / (10000.0 ** (np.arange(0, HD, 2, dtype=np.float64) / HD))
    invf64 = np.concatenate([inv_freq, inv_freq])          # [64]
    ang = invf64[:, None] * np.arange(T, dtype=np.float64)[None, :]  # [64, T]
    cos2 = np.tile(np.cos(ang), (2, 1)).astype(np.float32)
    sin2 = np.tile(np.sin(ang), (2, 1)).astype(np.float32)
    return np.ascontiguousarray(cos2), np.ascontiguousarray(sin2)


def kernel(x_real, x_imag, q_wr, q_wi, k_wr, k_wi, v_wr, v_wi, o_wr, o_wi):
    global _COMPILED, LAST_RESULTS
    if _COMPILED is None:
        _COMPILED = _build()
    nc = _COMPILED

    cos2, sin2 = _rope_tables()
    xt = {}
    for b in range(B):
        xt[("r", b)] = _to_bf16_kxm(np.asarray(x_real[b]).T.astype(np.float32))
        xt[("i", b)] = _to_bf16_kxm(np.asarray(x_imag[b]).T.astype(np.float32))

    in_maps = []
    for core in range(NCORE):
        b, g = core // TP, core % TP
        cols = slice(g * C, (g + 1) * C)
        m = {"xrT": xt[("r", b)], "xiT": xt[("i", b)],
             "cos2": cos2, "sin2": sin2}
        for nm, wr_, wi_ in (("wq", q_wr, q_wi), ("wk", k_wr, k_wi)):
            m[f"{nm}_r"] = _to_bf16_kxm(np.asarray(wr_[:, cols]))
            m[f"{nm}_i"] = _to_bf16_kxm(np.asarray(wi_[:, cols]))
            m[f"{nm}_n"] = _to_bf16_kxm(-np.asarray(wi_[:, cols]))
        vr_, vi_ = np.asarray(v_wr[:, cols]), np.asarray(v_wi[:, cols])
        m["wv_a"] = _to_bf16_kxm(np.concatenate([vr_, vi_], axis=1))
        m["wv_b"] = _to_bf16_kxm(np.concatenate([-vi_, vr_], axis=1))
        m["ow_r"] = _to_bf16_kxm(np.asarray(o_wr[cols, :]))
        m["ow_i"] = _to_bf16_kxm(np.asarray(o_wi[cols, :]))
        m["ow_n"] = _to_bf16_kxm(-np.asarray(o_wi[cols, :]))
        in_maps.append(m)

    res = run_bass_kernel_spmd(nc, in_maps, core_ids=list(range(NCORE)))
    LAST_RESULTS = res

    final_r = np.zeros((B, T, D), np.float32)
    final_i = np.zeros((B, T, D), np.float32)
    for core in range(NCORE):
        b = core // TP
        final_r[b] += res.results[core]["out_r"]
        final_i[b] += res.results[core]["out_i"]
    return final_r, final_i


# revision 19
# speedup vs baseline: 1.2728x; 1.0508x over previous
"""ComplexAttentionV3 Trainium2 kernel (v3).

Sharding: 8 cores = data-parallel over batch (2) x tensor-parallel over
heads (16 -> 4 per core). Each core computes q/k/v for its 4 heads
(column-sharded projections), local attention, and a row-sharded
o-projection producing a partial [T, D] output; the host sums the 4
partials per batch.

v3 notes vs v2 (559us baseline):
- attention software-pipelined: scores for pair p+1 are emitted before
  the AV matmuls of pair p, so the PE never idles waiting for the ACT
  exp (the 822ns/2jc stall + the resulting p-state down-clock were the
  dominant cost in v2's 272us attention phase);
- softmax denominator fused into the AV matmul as a 65th lhsT column of
  ones (v_real | 1), removing the separate dn matmuls' PSUM pool and
  freeing banks for double-buffered accumulators (av pools bufs=2), so
  the per-(h,iw) normalize chain (DVE recip + gpsimd broadcast + muls)
  runs off the PE critical path;
- attention works on 512-query windows: scores pair tile [128,1024]
  (2 banks, bufs=2) + avr [65,512] + avi [64,512] (1 bank, bufs=2 each)
  = 8 PSUM banks exactly;
- o-proj weights DMA'd at attention start (v2 left them to the o-proj
  phase and stalled 10us); outputs alternate between the sync and
  scalar HWDGE queues (v2 pushed all 16MB through one queue);
- x DMA'd in 512-column slabs interleaved r/i to match first-use order.
"""

import numpy as np
import ml_dtypes

import concourse.bacc as bacc
import concourse.tile as tile
from concourse import mybir
from concourse.bass import ts
from concourse.bass_utils import run_bass_kernel_spmd

B, T, D, H = 2, 2048, 1024, 16
HD = 64
NCORE = 8
TP = 4               # head-parallel degree (per batch)
HC = H // TP         # heads per core = 4
C = HC * HD          # local channels = 256
DC = D // 128        # contraction chunks = 8
TQ = T // 128        # 128-row t-chunks = 16
T5 = T // 512        # 512-col t-chunks = 4
TW = T // 1024       # 1024-col t-chunks = 2
NP = TQ // 2         # key-chunk pairs = 8

F32 = mybir.dt.float32
BF16 = mybir.dt.bfloat16
EXP = mybir.ActivationFunctionType.Exp

LAST_RESULTS = None
_COMPILED = None


def _build():
    nc = bacc.Bacc("TRN2", target_bir_lowering=False, debug=False,
                   num_devices=NCORE)

    def din(name, shape, dt=BF16):
        return nc.dram_tensor(name, shape, dt, kind="ExternalInput").ap()

    xr_d = din("xrT", [128, T5, DC, 512])
    xi_d = din("xiT", [128, T5, DC, 512])
    wq = {k: din(f"wq_{k}", [128, DC, C]) for k in ("r", "i", "n")}
    wk = {k: din(f"wk_{k}", [128, DC, C]) for k in ("r", "i", "n")}
    wv = {k: din(f"wv_{k}", [128, DC, 2 * C]) for k in ("a", "b")}
    ow = {k: din(f"ow_{k}", [128, 2, D]) for k in ("r", "i", "n")}
    cos_d = din("cos2", [128, T], F32)
    sin_d = din("sin2", [128, T], F32)
    outr_d = nc.dram_tensor("out_r", [T, D], F32, kind="ExternalOutput").ap()
    outi_d = nc.dram_tensor("out_i", [T, D], F32, kind="ExternalOutput").ap()

    with tile.TileContext(nc) as tc:
        with tc.tile_pool(name="persist", bufs=1) as persist:
            qkcat = persist.tile([128, 2 * HC, T], BF16, name="qkcat")
            # v_real and v_imag each padded to 65 columns per (key-chunk,
            # head): column 64 is ones. For v_real it computes the softmax
            # denominator into avr partition 64; for v_imag it only pads M
            # to 65 so the matmul stays in the PE's 128-column tile mode
            # (M=64 selects the 64-column mode and every mode switch costs
            # ~95ns). Flat index is tq * HC + h.
            vcr = persist.tile([128, TQ * HC, 65], BF16, name="vcr")
            vci = persist.tile([128, TQ * HC, 65], BF16, name="vci")
            # per-512-query-window u tiles (separate tiles so the o-proj's
            # reads don't pick up a false whole-tile dependency on the last
            # attention window)
            urts = [persist.tile([128, 2, 512], BF16, name=f"urt{w}")
                    for w in range(T5)]
            uits = [persist.tile([128, 2, 512], BF16, name=f"uit{w}")
                    for w in range(T5)]
            nc.vector.memset(vcr[:, :, 64:65], 1.0)
            nc.vector.memset(vci[:, :, 64:65], 1.0)

            # ---------------- projection phase ----------------
            with tc.tile_pool(name="xw", bufs=1) as xw, \
                 tc.tile_pool(name="rt", bufs=1) as rt, \
                 tc.tile_pool(name="pp", bufs=2, space="PSUM") as pp:
                # scalar HWDGE queue: weights + rope tables in first-use
                # order; sync HWDGE queue: x in 512-col slabs, r/i
                # interleaved (q-proj consumes slab s of xr then xi).
                wqs = {k: xw.tile([128, DC, C], BF16, name=f"wq{k}")
                       for k in ("r", "i", "n")}
                wks = {k: xw.tile([128, DC, C], BF16, name=f"wk{k}")
                       for k in ("r", "i", "n")}
                wvs = {k: xw.tile([128, DC, 2 * C], BF16, name=f"wv{k}")
                       for k in ("a", "b")}
                cos = xw.tile([128, T], F32, name="cos")
                sin = xw.tile([128, T], F32, name="sin")
                for k in ("r", "i", "n"):
                    nc.scalar.dma_start(wqs[k][:], wq[k][:])
                nc.scalar.dma_start(cos[:], cos_d[:])
                nc.scalar.dma_start(sin[:], sin_d[:])
                for k in ("r", "i", "n"):
                    nc.scalar.dma_start(wks[k][:], wk[k][:])
                for k in ("a", "b"):
                    nc.scalar.dma_start(wvs[k][:], wv[k][:])
                xr = xw.tile([128, T5, DC, 512], BF16, name="xr")
                xi = xw.tile([128, T5, DC, 512], BF16, name="xi")
                for sl in range(T5):
                    nc.sync.dma_start(xr[:, sl], xr_d[:, sl])
                    nc.sync.dma_start(xi[:, sl], xi_d[:, sl])

                # q/k projections (transposed [c, t]) + RoPE into qkcat
                for wsrc, hbase in ((wqs, 0), (wks, HC)):
                    for cc in range(2):
                        h0, h1 = hbase + 2 * cc, hbase + 2 * cc + 1
                        for tw in range(TW):
                            pqr = pp.tile([128, 1024], F32, name="ppa")
                            pqi = pp.tile([128, 1024], F32, name="ppb")
                            for half in range(2):
                                sl = 2 * tw + half
                                psl = ts(half, 512)
                                for dc in range(DC):
                                    nc.tensor.matmul(
                                        pqr[:, psl],
                                        lhsT=wsrc["r"][:, dc, ts(cc, 128)],
                                        rhs=xr[:, sl, dc, :],
                                        start=(dc == 0), stop=False)
                                for dc in range(DC):
                                    nc.tensor.matmul(
                                        pqr[:, psl],
                                        lhsT=wsrc["n"][:, dc, ts(cc, 128)],
                                        rhs=xi[:, sl, dc, :],
                                        start=False, stop=(dc == DC - 1))
                                for dc in range(DC):
                                    nc.tensor.matmul(
                                        pqi[:, psl],
                                        lhsT=wsrc["i"][:, dc, ts(cc, 128)],
                                        rhs=xr[:, sl, dc, :],
                                        start=(dc == 0), stop=False)
                                for dc in range(DC):
                                    nc.tensor.matmul(
                                        pqi[:, psl],
                                        lhsT=wsrc["r"][:, dc, ts(cc, 128)],
                                        rhs=xi[:, sl, dc, :],
                                        start=False, stop=(dc == DC - 1))
                            tsl = ts(tw, 1024)
                            t1 = rt.tile([128, 1024], F32, name="t1")
                            t2 = rt.tile([128, 1024], F32, name="t2")
                            t3 = rt.tile([128, 1024], F32, name="t3")
                            t4 = rt.tile([128, 1024], F32, name="t4")
                            nc.vector.tensor_mul(t1[:], pqr[:], cos[:, tsl])
                            nc.vector.tensor_mul(t2[:], pqi[:], sin[:, tsl])
                            nc.vector.tensor_mul(t3[:], pqr[:], sin[:, tsl])
                            nc.vector.tensor_mul(t4[:], pqi[:], cos[:, tsl])
                            nc.vector.tensor_sub(qkcat[0:64, h0, tsl],
                                                 t1[0:64, :], t2[0:64, :])
                            nc.vector.tensor_sub(qkcat[0:64, h1, tsl],
                                                 t1[64:128, :], t2[64:128, :])
                            nc.vector.tensor_add(qkcat[64:128, h0, tsl],
                                                 t3[0:64, :], t4[0:64, :])
                            nc.vector.tensor_add(qkcat[64:128, h1, tsl],
                                                 t3[64:128, :], t4[64:128, :])

                # v projection: natural [t, c], rhs packed [wvr | wvi]
                for tq in range(TQ):
                    pv = pp.tile([128, 1024], F32, name="ppa")
                    pvs = pv[:, 0:512]
                    w, off = tq // 4, (tq % 4) * 128
                    for dc in range(DC):
                        nc.tensor.matmul(pvs,
                                         lhsT=xr[:, w, dc, off:off + 128],
                                         rhs=wvs["a"][:, dc, :],
                                         start=(dc == 0), stop=False)
                    for dc in range(DC):
                        nc.tensor.matmul(pvs,
                                         lhsT=xi[:, w, dc, off:off + 128],
                                         rhs=wvs["b"][:, dc, :],
                                         start=False, stop=(dc == DC - 1))
                    nc.scalar.copy(
                        vcr[:, tq * HC:(tq + 1) * HC, 0:64],
                        pv[:, 0:C].rearrange("p (h d) -> p h d", h=HC))
                    nc.scalar.copy(
                        vci[:, tq * HC:(tq + 1) * HC, 0:64],
                        pv[:, C:2 * C].rearrange("p (h d) -> p h d", h=HC))

            # ---------------- attention phase ----------------
            # per (head, 512-query window): 8 key-chunk pairs; scores for
            # pair p+1 are emitted before the AV matmuls of pair p so the
            # exp latency is hidden behind ~1.3us of PE work.
            with tc.tile_pool(name="ox", bufs=1) as ox:
                # prefetch o-proj weights now: the scalar queue is idle and
                # SBUF has room once the projection pools wind down.
                ows = {k: ox.tile([128, 2, D], BF16, name=f"ow{k}")
                       for k in ("r", "i", "n")}
                for k in ("r", "i", "n"):
                    nc.scalar.dma_start(ows[k][:], ow[k][:])

                with tc.tile_pool(name="att", bufs=3) as att, \
                     tc.tile_pool(name="attsm", bufs=2) as attsm, \
                     tc.tile_pool(name="sp", bufs=2, space="PSUM") as sp, \
                     tc.tile_pool(name="avr", bufs=2, space="PSUM") as avrp, \
                     tc.tile_pool(name="avi", bufs=2, space="PSUM") as avip:
                    for iw in range(T5):
                        isl = ts(iw, 512)
                        for h in range(HC):
                            ucc, up0 = h // 2, (h % 2) * 64
                            qsl = qkcat[:, h, isl]
                            avr = avrp.tile([65, 512], F32, name="avr")
                            avi = avip.tile([65, 512], F32, name="avi")

                            def scores(p, etiles, h=h, qsl=qsl):
                                s = sp.tile([128, 1024], F32, name="s")
                                for j in range(2):
                                    nc.tensor.matmul(
                                        s[:, ts(j, 512)],
                                        lhsT=qkcat[:, HC + h,
                                                   ts(2 * p + j, 128)],
                                        rhs=qsl, start=True, stop=True)
                                es = att.tile([128, 1024], BF16, name="es")
                                nc.scalar.activation(es[:], s[:], EXP,
                                                     scale=0.125)
                                etiles[p] = es

                            def accum(p, etiles, h=h, avr=avr, avi=avi):
                                es = etiles.pop(p)
                                for j in range(2):
                                    nc.tensor.matmul(
                                        avr[:],
                                        lhsT=vcr[:, (2 * p + j) * HC + h, :],
                                        rhs=es[:, ts(j, 512)],
                                        start=(p == 0 and j == 0),
                                        stop=(p == NP - 1 and j == 1))
                                for j in range(2):
                                    nc.tensor.matmul(
                                        avi[:],
                                        lhsT=vci[:, (2 * p + j) * HC + h, :],
                                        rhs=es[:, ts(j, 512)],
                                        start=(p == 0 and j == 0),
                                        stop=(p == NP - 1 and j == 1))

                            etiles = {}
                            scores(0, etiles)
                            for p in range(NP):
                                if p + 1 < NP:
                                    scores(p + 1, etiles)
                                accum(p, etiles)

                            dnr = attsm.tile([1, 512], F32, name="dnr")
                            nc.scalar.copy(dnr[:], avr[64:65, :])
                            rec = attsm.tile([1, 512], F32, name="rec")
                            nc.vector.reciprocal_approx_fast(rec[:], dnr[:])
                            bc = attsm.tile([128, 512], F32, name="bc")
                            nc.gpsimd.partition_broadcast(bc[:], rec[:])
                            nc.vector.tensor_mul(urts[iw][up0:up0 + 64, ucc, :],
                                                 avr[0:64, :], bc[0:64, :])
                            nc.vector.tensor_mul(uits[iw][up0:up0 + 64, ucc, :],
                                                 avi[0:64, :], bc[64:128, :])

                # ---------------- output projection ----------------
                with tc.tile_pool(name="ost", bufs=3) as ost, \
                     tc.tile_pool(name="op", bufs=2, space="PSUM") as op:
                    for tq in range(TQ):
                        tslq = ts(tq, 128)
                        w, off = tq // 4, (tq % 4) * 128
                        ur, ui = urts[w], uits[w]
                        usl = slice(off, off + 128)
                        por = op.tile([128, 1024], F32, name="opa")
                        poi = op.tile([128, 1024], F32, name="opb")
                        for oc in range(2):
                            osl = ts(oc, 512)
                            nc.tensor.matmul(por[:, osl], lhsT=ur[:, 0, usl],
                                             rhs=ows["r"][:, 0, osl],
                                             start=True, stop=False)
                            nc.tensor.matmul(por[:, osl], lhsT=ur[:, 1, usl],
                                             rhs=ows["r"][:, 1, osl],
                                             start=False, stop=False)
                            nc.tensor.matmul(por[:, osl], lhsT=ui[:, 0, usl],
                                             rhs=ows["n"][:, 0, osl],
                                             start=False, stop=False)
                            nc.tensor.matmul(por[:, osl], lhsT=ui[:, 1, usl],
                                             rhs=ows["n"][:, 1, osl],
                                             start=False, stop=True)
                            nc.tensor.matmul(poi[:, osl], lhsT=ur[:, 0, usl],
                                             rhs=ows["i"][:, 0, osl],
                                             start=True, stop=False)
                            nc.tensor.matmul(poi[:, osl], lhsT=ur[:, 1, usl],
                                             rhs=ows["i"][:, 1, osl],
                                             start=False, stop=False)
                            nc.tensor.matmul(poi[:, osl], lhsT=ui[:, 0, usl],
                                             rhs=ows["r"][:, 0, osl],
                                             start=False, stop=False)
                            nc.tensor.matmul(poi[:, osl], lhsT=ui[:, 1, usl],
                                             rhs=ows["r"][:, 1, osl],
                                             start=False, stop=True)
                        st = ost.tile([128, 1024], F32, name="st")
                        nc.scalar.copy(st[:], por[:])
                        nc.sync.dma_start(outr_d[tslq, :], st[:])
                        sti = ost.tile([128, 1024], F32, name="sti")
                        nc.vector.tensor_copy(sti[:], poi[:])
                        nc.scalar.dma_start(outi_d[tslq, :], sti[:])

    nc.compile()
    return nc


def _to_bf16_kxm(arr, parts=128):
    """[K, M] fp32 -> [128, K//128, M] bf16 with K split as (chunk, part)."""
    k, m = arr.shape
    out = arr.reshape(k // parts, parts, m).transpose(1, 0, 2)
    return np.ascontiguousarray(out.astype(ml_dtypes.bfloat16))


def _to_x_slabs(arr):
    """[T, D] fp32 -> [128, T5, DC, 512] bf16: D split as (chunk, part),
    T split into 512-col slabs, slab-major so each slab is contiguous."""
    out = _to_bf16_kxm(arr.T.astype(np.float32))        # [128, DC, T]
    out = out.reshape(128, DC, T5, 512).transpose(0, 2, 1, 3)
    return np.ascontiguousarray(out)


def _rope_tables():
    inv_freq = 1.0 / (10000.0 ** (np.arange(0, HD, 2, dtype=np.float64) / HD))
    invf64 = np.concatenate([inv_freq, inv_freq])          # [64]
    ang = invf64[:, None] * np.arange(T, dtype=np.float64)[None, :]  # [64, T]
    cos2 = np.tile(np.cos(ang), (2, 1)).astype(np.float32)
    sin2 = np.tile(np.sin(ang), (2, 1)).astype(np.float32)
    return np.ascontiguousarray(cos2), np.ascontiguousarray(sin2)


def kernel(x_real, x_imag, q_wr, q_wi, k_wr, k_wi, v_wr, v_wi, o_wr, o_wi):
    global _COMPILED, LAST_RESULTS
    if _COMPILED is None:
        _COMPILED = _build()
    nc = _COMPILED

    cos2, sin2 = _rope_tables()
    xt = {}
    for b in range(B):
        xt[("r", b)] = _to_x_slabs(np.asarray(x_real[b]))
        xt[("i", b)] = _to_x_slabs(np.asarray(x_imag[b]))

    in_maps = []
    for core in range(NCORE):
        b, g = core // TP, core % TP
        cols = slice(g * C, (g + 1) * C)
        m = {"xrT": xt[("r", b)], "xiT": xt[("i", b)],
             "cos2": cos2, "sin2": sin2}
        for nm, wr_, wi_ in (("wq", q_wr, q_wi), ("wk", k_wr, k_wi)):
            m[f"{nm}_r"] = _to_bf16_kxm(np.asarray(wr_[:, cols]))
            m[f"{nm}_i"] = _to_bf16_kxm(np.asarray(wi_[:, cols]))
            m[f"{nm}_n"] = _to_bf16_kxm(-np.asarray(wi_[:, cols]))
        vr_, vi_ = np.asarray(v_wr[:, cols]), np.asarray(v_wi[:, cols])
        m["wv_a"] = _to_bf16_kxm(np.concatenate([vr_, vi_], axis=1))
        m["wv_b"] = _to_bf16_kxm(np.concatenate([-vi_, vr_], axis=1))
        m["ow_r"] = _to_bf16_kxm(np.asarray(o_wr[cols, :]))
        m["ow_i"] = _to_bf16_kxm(np.asarray(o_wi[cols, :]))
        m["ow_n"] = _to_bf16_kxm(-np.asarray(o_wi[cols, :]))
        in_maps.append(m)

    res = run_bass_kernel_spmd(nc, in_maps, core_ids=list(range(NCORE)))
    LAST_RESULTS = res

    final_r = np.zeros((B, T, D), np.float32)
    final_i = np.zeros((B, T, D), np.float32)
    for core in range(NCORE):
        b = core // TP
        final_r[b] += res.results[core]["out_r"]
        final_i[b] += res.results[core]["out_i"]
    return final_r, final_i
